# revision 16
# baseline (speedup 1.0000x reference)
"""Sparse-attention Trainium2 kernel (nn_AttentionLayer, B=16 S=2048 D=128).

reference semantics:
    A = Q @ T^T                     # [B,S,S]
    A = where(A > 0.3, A, 0)
    A += where(strictly_upper, -2^32, 0)
    y = softmax(A / sqrt(D)) @ V

Sharding: data-parallel over batch, 2 batches per core on 8 NeuronCores.
No collectives.

v2 design (per core, per batch):
  - Loads split in chunks and issued up front; prep (PE transposes of
    Q/T into [128,1024] PSUM groups + one big f32->bf16 cast-copy per
    group on DVE) is pipelined under the main loop; batch-1 prep is
    emitted inside batch-0's qb loop so PE/DVE never idle.
  - Scores S^T[k,q] via matmul(lhsT=tT tile, rhs=qT block) in bf16.
    Exact-causal: off-diagonal k-tile pairs in [128,1024] PSUM groups;
    the 4 diagonal k-tiles are column-trimmed (512/384/256/128 cols).
  - num = max(exp(S^T/sqrt(d)), 1): exp on ScalarE (scale fused),
    max on DVE in bf16 (4x perf mode). Equals the reference's
    threshold-then-exp except on scores in (0,0.3], error <=2.7% of
    one softmax term.
  - Causal wedge: one [128,128] affine_select per diagonal k-tile on
    GpSimd (fill=0), only on the 128 columns straddling the diagonal.
  - PV + denominator fused: lhsT = num chunk [k,128q], rhs =
    [V | ones] [k,129] bf16, accumulated in PSUM over k. obanks packed
    2 subtiles per PSUM tile [128,2,129]. Denominator at column 128.
  - Normalize: obank pair copied PSUM->SBUF on DVE, then
    normalize_recip (out = pv/den) on GpSimd. Stores 1 per 2 q-blocks.
"""

from contextlib import ExitStack

import numpy as np

import concourse.bass as bass
import concourse.mybir as mybir
import concourse.tile as tile
from concourse import bacc

B, S, D = 16, 2048, 128
N_CORES = 8
B_LOC = B // N_CORES          # 2 batches per core
QB = 512                      # q-block width (matmul moving dim)
KT = 128                      # k-tile height (partition dim)
N_QB = S // QB                # 4 q-blocks
N_ST = S // 128               # 16 seq tiles
SCALE = float(1.0 / np.sqrt(D))

F32 = mybir.dt.float32
BF16 = mybir.dt.bfloat16


def build_attention_core():
    """Build the single-core SPMD graph: [B_LOC,S,D] Q/T/V -> [B_LOC,S,D] out."""
    from concourse.masks import make_identity

    nc = bacc.Bacc("TRN2", target_bir_lowering=False, debug=False,
                   num_devices=N_CORES)
    q_ext = nc.dram_tensor("Q", [B_LOC, S, D], F32, kind="ExternalInput").ap()
    t_ext = nc.dram_tensor("T", [B_LOC, S, D], F32, kind="ExternalInput").ap()
    v_ext = nc.dram_tensor("V", [B_LOC, S, D], F32, kind="ExternalInput").ap()
    o_ext = nc.dram_tensor("out", [B_LOC, S, D], F32, kind="ExternalOutput").ap()

    with tile.TileContext(nc) as tc, ExitStack() as ctx:
        nat_pool = ctx.enter_context(tc.tile_pool(name="nat", bufs=1))
        qt_pool = ctx.enter_context(tc.tile_pool(name="qt", bufs=1))
        tt_pool = ctx.enter_context(tc.tile_pool(name="tt", bufs=1))
        vb_pool = ctx.enter_context(tc.tile_pool(name="vb", bufs=1))
        num_pool = ctx.enter_context(tc.tile_pool(name="num", bufs=4))
        stg_pool = ctx.enter_context(tc.tile_pool(name="stg", bufs=2))
        fin_pool = ctx.enter_context(tc.tile_pool(name="fin", bufs=2))
        const_pool = ctx.enter_context(tc.tile_pool(name="const", bufs=1))
        # PSUM: qk pool tiles [128,1024] f32 = 2 banks x2 bufs = 4 banks;
        # out pool tiles [128,2,129] f32 = 1 bank x4 bufs = 4 banks.
        qk_psum = ctx.enter_context(tc.tile_pool(name="qk_ps", bufs=2, space="PSUM"))
        out_psum = ctx.enter_context(tc.tile_pool(name="out_ps", bufs=1, space="PSUM"))

        ident = const_pool.tile([128, 128], F32)
        make_identity(nc, ident[:])

        # ---- issue ALL input loads up front (b0 in halves for fast start) ----
        nats = []   # per batch: (q_nat, t_nat, v_nat)
        for b in range(B_LOC):
            q_nat = nat_pool.tile([128, N_ST, D], F32, name=f"q_nat{b}")
            t_nat = nat_pool.tile([128, N_ST, D], F32, name=f"t_nat{b}")
            v_nat = nat_pool.tile([128, N_ST, D], F32, name=f"v_nat{b}")
            nats.append((q_nat, t_nat, v_nat))
        # batch-0 loads in halves on the Sync queue, T before Q (transposes
        # consume T first), V last. Batch-1 loads are issued later from the
        # GpSimd queue (see the main loop) so Sync's issue cost doesn't
        # delay batch-0's pipeline fill.
        exts = [(t_ext, 1), (q_ext, 0), (v_ext, 2)]
        for half in range(2):
            t0, t1 = half * 8, half * 8 + 8
            for ext, which in exts:
                nc.sync.dma_start(
                    nats[0][which][:, t0:t1, :],
                    ext[0].rearrange("(t p) d -> p t d", p=128)[:, t0:t1, :])

        def emit_b1_loads():
            for ext, which in exts:
                nc.gpsimd.dma_start(
                    nats[1][which][:],
                    ext[1].rearrange("(t p) d -> p t d", p=128))

        # ---- prep helpers ------------------------------------------------
        qTs, tTs, v_augs = {}, {}, {}

        def alloc_prep(b):
            qTs[b] = qt_pool.tile([128, N_ST, 128], BF16, name=f"qT{b}")
            tTs[b] = tt_pool.tile([128, N_ST, 128], BF16, name=f"tT{b}")
            v_augs[b] = vb_pool.tile([128, N_ST, 132], BF16, name=f"vaug{b}")

        def emit_transpose_group(b, which, t0):
            """Transpose 8 tiles of Q (which=0) or T (which=1) of batch b,
            tiles [t0, t0+8), into one [128,1024] PSUM group, then one
            f32->bf16 cast-copy to the persistent qT/tT tile."""
            src = nats[b][which]
            dst = (qTs if which == 0 else tTs)[b]
            ps = qk_psum.tile([128, 1024], F32, tag="qk")
            for i in range(8):
                nc.tensor.transpose(ps[:, i * 128:(i + 1) * 128],
                                    src[:, t0 + i, :], ident[:])
            nc.vector.tensor_copy(dst[:, t0:t0 + 8, :], ps[:])

        def emit_vaug(b, half, eng):
            """Cast half of V to bf16 into v_aug on the given engine."""
            t0, t1 = half * 8, half * 8 + 8
            eng.tensor_copy(v_augs[b][:, t0:t1, 0:D], nats[b][2][:, t0:t1, :])

        # ---- batch-0 prep ----
        alloc_prep(0)
        emit_transpose_group(0, 1, 0)   # T first half (needed by qb0 scores)
        emit_transpose_group(0, 0, 0)   # Q first half
        emit_vaug(0, 0, nc.vector)
        nc.gpsimd.memset(v_augs[0][:, :, D:D + 1], 1.0)
        emit_transpose_group(0, 1, 8)
        emit_transpose_group(0, 0, 8)
        emit_vaug(0, 1, nc.vector)

        # deferred prep work for batch 1, interleaved into batch-0 main loop
        def prep_b1_first():
            alloc_prep(1)
            emit_transpose_group(1, 1, 0)
            emit_transpose_group(1, 0, 0)
            emit_vaug(1, 0, nc.vector)
            nc.gpsimd.memset(v_augs[1][:, :, D:D + 1], 1.0)

        def prep_b1_second():
            emit_transpose_group(1, 1, 8)
            emit_transpose_group(1, 0, 8)
            emit_vaug(1, 1, nc.vector)

        # ---- main attention loops ----
        def emit_qb(b, qb, fin):
            """Emit one q-block: scores -> exp/max/select -> PV -> normalize.
            fin: [128, 8, 128] staging tile for 2 q-blocks; this qb uses
            slot (qb % 2)."""
            qT_flat = qTs[b][:].rearrange("p t q -> p (t q)")
            tT_flat = tTs[b][:].rearrange("p t k -> p (t k)")
            v_aug = v_augs[b]
            q0 = qb * QB
            c_diag = 4 * qb              # first diagonal k-tile index

            # one PSUM tile, 4 bank-aligned sub-accumulators [128, 129 of 512]
            obank = out_psum.tile([128, 4, 512], F32, tag="ob")

            def pv(c, num_ap, subs):
                """PV matmuls for k-tile c; num_ap[:, i*128:(i+1)*128] is the
                numerator chunk for sub subs[i]."""
                for i, sub in enumerate(subs):
                    nc.tensor.matmul(
                        obank[:, sub, 0:129],
                        lhsT=num_ap[:, i * 128:(i + 1) * 128],
                        rhs=v_aug[:, c, 0:129],
                        start=(c == 0),
                        stop=(c == c_diag + sub),
                    )

            # off-diagonal full k-tile pairs
            for g in range(c_diag // 2):
                cs = (2 * g, 2 * g + 1)
                s_ps = qk_psum.tile([128, 1024], F32, tag="qk")
                for j, c in enumerate(cs):
                    nc.tensor.matmul(
                        s_ps[:, j * 512:(j + 1) * 512],
                        lhsT=tT_flat[:, c * KT:(c + 1) * KT],
                        rhs=qT_flat[:, q0:q0 + QB],
                    )
                num = num_pool.tile([128, 1024], BF16)
                nc.scalar.activation(num[:], s_ps[:],
                                     mybir.ActivationFunctionType.Exp,
                                     scale=SCALE)
                nc.vector.tensor_scalar_max(num[:], num[:], 1.0)
                for j, c in enumerate(cs):
                    pv(c, num[:, j * 512:(j + 1) * 512], (0, 1, 2, 3))

            # diagonal block: k-tiles c_diag+j, trimmed to 512-128j columns,
            # packed in two PSUM groups: (j=0: 512, j=1: 384) and
            # (j=2: 256, j=3: 128).
            for grp, js in enumerate(((0, 1), (2, 3))):
                widths = [QB - 128 * j for j in js]
                s_ps = qk_psum.tile([128, 1024], F32, tag="qk")
                off = 0
                offs = []
                for j, w in zip(js, widths):
                    nc.tensor.matmul(
                        s_ps[:, off:off + w],
                        lhsT=tT_flat[:, (c_diag + j) * KT:(c_diag + j + 1) * KT],
                        rhs=qT_flat[:, q0 + 128 * j:q0 + QB],
                    )
                    offs.append(off)
                    off += w
                num = num_pool.tile([128, 1024], BF16)
                nc.scalar.activation(num[:, 0:off], s_ps[:, 0:off],
                                     mybir.ActivationFunctionType.Exp,
                                     scale=SCALE)
                nc.vector.tensor_scalar_max(num[:, 0:off], num[:, 0:off], 1.0)
                # causal wedge: first 128 computed cols of each diagonal tile
                for j, o in zip(js, offs):
                    nc.gpsimd.affine_select(
                        out=num[:, o:o + 128],
                        in_=num[:, o:o + 128],
                        compare_op=mybir.AluOpType.is_ge,
                        fill=0.0,
                        base=0,
                        channel_multiplier=-1,
                        pattern=[[1, 128]],
                    )
                for j, o, w in zip(js, offs, widths):
                    pv(c_diag + j, num[:, o:o + w], tuple(range(j, 4)))

            # ---- normalize: PSUM->SBUF copy (DVE), pv/den on GpSimd ----
            stg = stg_pool.tile([128, 4, 129], F32, tag="stg")
            nc.vector.tensor_copy(stg[:], obank[:, :, 0:129])
            for sub in range(4):
                nc.gpsimd.normalize_recip(
                    fin[:, (qb % 2) * 4 + sub, :],
                    stg[:, sub, 0:D],
                    stg[:, sub, D:D + 1],
                )

        for b in range(B_LOC):
            for qb in range(N_QB):
                if qb % 2 == 0:
                    fin = fin_pool.tile([128, 8, 128], F32, tag="fin")
                emit_qb(b, qb, fin)
                if qb % 2 == 1:
                    q0 = (qb - 1) * QB
                    nc.sync.dma_start(
                        o_ext[b, q0:q0 + 2 * QB, :].rearrange(
                            "(s p) d -> p s d", p=128),
                        fin[:])
                # interleave batch-1 loads and prep into batch 0's q-blocks
                if b == 0 and qb == 0:
                    emit_b1_loads()
                if b == 0 and qb == 2:
                    prep_b1_first()
                if b == 0 and qb == 3:
                    prep_b1_second()

    nc.compile()
    return nc


_NC_CACHE = None


def _get_nc():
    global _NC_CACHE
    if _NC_CACHE is None:
        _NC_CACHE = build_attention_core()
    return _NC_CACHE


def kernel(Q: np.ndarray, T: np.ndarray, V: np.ndarray) -> np.ndarray:
    """Full-input entry point: shard over batch, run 8-core SPMD, gather."""
    from concourse.bass_utils import run_bass_kernel_spmd

    Q = np.ascontiguousarray(np.asarray(Q, dtype=np.float32))
    T = np.ascontiguousarray(np.asarray(T, dtype=np.float32))
    V = np.ascontiguousarray(np.asarray(V, dtype=np.float32))
    assert Q.shape == (B, S, D), Q.shape

    nc = _get_nc()
    in_maps = [
        {
            "Q": Q[i * B_LOC:(i + 1) * B_LOC],
            "T": T[i * B_LOC:(i + 1) * B_LOC],
            "V": V[i * B_LOC:(i + 1) * B_LOC],
        }
        for i in range(N_CORES)
    ]
    res = run_bass_kernel_spmd(nc, in_maps, core_ids=list(range(N_CORES)))
    return np.concatenate([res.results[i]["out"] for i in range(N_CORES)], axis=0)


# revision 23
# speedup vs baseline: 1.1181x; 1.1181x over previous
"""Sparse-attention Trainium2 kernel (nn_AttentionLayer, B=16 S=2048 D=128).

reference semantics:
    A = Q @ T^T                     # [B,S,S]
    A = where(A > 0.3, A, 0)
    A += where(strictly_upper, -2^32, 0)
    y = softmax(A / sqrt(D)) @ V

Sharding: data-parallel over batch, 2 batches per core on 8 NeuronCores.
No collectives.

v2 design (per core, per batch):
  - Loads split in chunks and issued up front; prep (PE transposes of
    Q/T into [128,1024] PSUM groups + one big f32->bf16 cast-copy per
    group on DVE) is pipelined under the main loop; batch-1 prep is
    emitted inside batch-0's qb loop so PE/DVE never idle.
  - Scores S^T[k,q] via matmul(lhsT=tT tile, rhs=qT block) in bf16.
    Exact-causal: off-diagonal k-tile pairs in [128,1024] PSUM groups;
    the 4 diagonal k-tiles are column-trimmed (512/384/256/128 cols).
  - num = max(exp(S^T/sqrt(d)), 1): exp on ScalarE (scale fused),
    max on DVE in bf16 (4x perf mode). Equals the reference's
    threshold-then-exp except on scores in (0,0.3], error <=2.7% of
    one softmax term.
  - Causal wedge: one [128,128] affine_select per diagonal k-tile on
    GpSimd (fill=0), only on the 128 columns straddling the diagonal.
  - PV + denominator fused: lhsT = num chunk [k,128q], rhs =
    [V | ones] [k,129] bf16, accumulated in PSUM over k. obanks packed
    2 subtiles per PSUM tile [128,2,129]. Denominator at column 128.
  - Normalize: obank pair copied PSUM->SBUF on DVE, then
    normalize_recip (out = pv/den) on GpSimd. Stores 1 per 2 q-blocks.
"""

from contextlib import ExitStack

import numpy as np

import concourse.bass as bass
import concourse.mybir as mybir
import concourse.tile as tile
from concourse import bacc

B, S, D = 16, 2048, 128
N_CORES = 8
B_LOC = B // N_CORES          # 2 batches per core
QB = 512                      # q-block width (matmul moving dim)
KT = 128                      # k-tile height (partition dim)
N_QB = S // QB                # 4 q-blocks
N_ST = S // 128               # 16 seq tiles
SCALE = float(1.0 / np.sqrt(D))

F32 = mybir.dt.float32
BF16 = mybir.dt.bfloat16


def build_attention_core():
    """Build the single-core SPMD graph: [B_LOC,S,D] Q/T/V -> [B_LOC,S,D] out."""
    from concourse.masks import make_identity

    nc = bacc.Bacc("TRN2", target_bir_lowering=False, debug=False,
                   num_devices=N_CORES)
    q_ext = nc.dram_tensor("Q", [B_LOC, S, D], F32, kind="ExternalInput").ap()
    t_ext = nc.dram_tensor("T", [B_LOC, S, D], F32, kind="ExternalInput").ap()
    v_ext = nc.dram_tensor("V", [B_LOC, S, D], F32, kind="ExternalInput").ap()
    o_ext = nc.dram_tensor("out", [B_LOC, S, D], F32, kind="ExternalOutput").ap()

    with tile.TileContext(nc) as tc, ExitStack() as ctx:
        nat_pool = ctx.enter_context(tc.tile_pool(name="nat", bufs=1))
        qt_pool = ctx.enter_context(tc.tile_pool(name="qt", bufs=1))
        tt_pool = ctx.enter_context(tc.tile_pool(name="tt", bufs=1))
        vb_pool = ctx.enter_context(tc.tile_pool(name="vb", bufs=1))
        num_pool = ctx.enter_context(tc.tile_pool(name="num", bufs=4))
        stg_pool = ctx.enter_context(tc.tile_pool(name="stg", bufs=2))
        fin_pool = ctx.enter_context(tc.tile_pool(name="fin", bufs=2))
        const_pool = ctx.enter_context(tc.tile_pool(name="const", bufs=1))
        # PSUM: qk pool tiles [128,1024] f32 = 2 banks x2 bufs = 4 banks;
        # out pool tiles [128,2,129] f32 = 1 bank x4 bufs = 4 banks.
        qk_psum = ctx.enter_context(tc.tile_pool(name="qk_ps", bufs=2, space="PSUM"))
        out_psum = ctx.enter_context(tc.tile_pool(name="out_ps", bufs=1, space="PSUM"))

        ident = const_pool.tile([128, 128], F32)
        make_identity(nc, ident[:])

        # ---- issue ALL input loads up front (b0 in halves for fast start) ----
        nats = []   # per batch: (q_nat, t_nat, v_nat)
        for b in range(B_LOC):
            q_nat = nat_pool.tile([128, N_ST, D], F32, name=f"q_nat{b}")
            t_nat = nat_pool.tile([128, N_ST, D], F32, name=f"t_nat{b}")
            v_nat = nat_pool.tile([128, N_ST, D], F32, name=f"v_nat{b}")
            nats.append((q_nat, t_nat, v_nat))
        # batch-0 loads in halves, issue-parallelized across engine queues so
        # the ~1.3us SWDGE cost per dma_start doesn't serialize the fill.
        # Batch-1 loads are issued later from the GpSimd queue (main loop).
        exts = [(t_ext, 1), (q_ext, 0), (v_ext, 2)]

        def load0(eng, ei, half):
            ext, which = exts[ei]
            t0, t1 = half * 8, half * 8 + 8
            eng.dma_start(
                nats[0][which][:, t0:t1, :],
                ext[0].rearrange("(t p) d -> p t d", p=128)[:, t0:t1, :])

        load0(nc.sync, 0, 0)     # T first half — gates the first transposes
        load0(nc.scalar, 1, 0)   # Q first half
        load0(nc.gpsimd, 2, 0)   # V first half
        load0(nc.sync, 0, 1)     # T second half
        load0(nc.scalar, 1, 1)   # Q second half
        load0(nc.sync, 2, 1)     # V second half

        def emit_b1_loads():
            for ext, which in exts:
                nc.gpsimd.dma_start(
                    nats[1][which][:],
                    ext[1].rearrange("(t p) d -> p t d", p=128))

        # ---- prep helpers ------------------------------------------------
        qTs, tTs, v_augs = {}, {}, {}

        def alloc_prep(b):
            qTs[b] = qt_pool.tile([128, N_ST, 128], BF16, name=f"qT{b}")
            tTs[b] = tt_pool.tile([128, N_ST, 128], BF16, name=f"tT{b}")
            v_augs[b] = vb_pool.tile([128, N_ST, 132], BF16, name=f"vaug{b}")

        def emit_transpose_group(b, which, t0):
            """Transpose 8 tiles of Q (which=0) or T (which=1) of batch b,
            tiles [t0, t0+8), into one [128,1024] PSUM group, then one
            f32->bf16 cast-copy to the persistent qT/tT tile."""
            src = nats[b][which]
            dst = (qTs if which == 0 else tTs)[b]
            ps = qk_psum.tile([128, 1024], F32, tag="qk")
            for i in range(8):
                nc.tensor.transpose(ps[:, i * 128:(i + 1) * 128],
                                    src[:, t0 + i, :], ident[:])
            nc.vector.tensor_copy(dst[:, t0:t0 + 8, :], ps[:])

        def emit_vaug(b, half, eng):
            """Cast half of V to bf16 into v_aug on the given engine."""
            t0, t1 = half * 8, half * 8 + 8
            eng.tensor_copy(v_augs[b][:, t0:t1, 0:D], nats[b][2][:, t0:t1, :])

        # ---- batch-0 prep ----
        alloc_prep(0)
        emit_transpose_group(0, 1, 0)   # T first half (needed by qb0 scores)
        emit_transpose_group(0, 0, 0)   # Q first half
        emit_vaug(0, 0, nc.vector)
        nc.gpsimd.memset(v_augs[0][:, :, D:D + 1], 1.0)
        emit_transpose_group(0, 1, 8)
        emit_transpose_group(0, 0, 8)
        emit_vaug(0, 1, nc.vector)

        # deferred prep work for batch 1, interleaved between score groups of
        # batch 0's later q-blocks (each hook is one small unit of work so no
        # single insertion starves the exp pipeline)
        b1_hooks_qb3 = [
            lambda: emit_transpose_group(1, 1, 0),
            lambda: emit_transpose_group(1, 0, 0),
            lambda: (emit_vaug(1, 0, nc.vector),
                     nc.gpsimd.memset(v_augs[1][:, :, D:D + 1], 1.0)),
        ]
        b1_hooks_late = [
            lambda: emit_transpose_group(1, 1, 8),
            lambda: emit_transpose_group(1, 0, 8),
            lambda: emit_vaug(1, 1, nc.vector),
        ]

        # ---- main attention loops ----
        def emit_qb(b, qb, fin, hooks=()):
            """Emit one q-block: scores -> exp/max/select -> PV -> normalize.
            fin: [128, 8, 128] staging tile for 2 q-blocks; this qb uses
            slot (qb % 2). hooks: thunks emitted one-per-score-group to
            interleave prep work for the next batch."""
            hooks = list(hooks)
            qT_flat = qTs[b][:].rearrange("p t q -> p (t q)")
            tT_flat = tTs[b][:].rearrange("p t k -> p (t k)")
            v_aug = v_augs[b]
            q0 = qb * QB
            c_diag = 4 * qb              # first diagonal k-tile index

            # one PSUM tile, 4 bank-aligned sub-accumulators [128, 129 of 512]
            obank = out_psum.tile([128, 4, 512], F32, tag="ob")

            def pv(c, num_ap, subs):
                """PV matmuls for k-tile c; num_ap[:, i*128:(i+1)*128] is the
                numerator chunk for sub subs[i]."""
                for i, sub in enumerate(subs):
                    nc.tensor.matmul(
                        obank[:, sub, 0:129],
                        lhsT=num_ap[:, i * 128:(i + 1) * 128],
                        rhs=v_aug[:, c, 0:129],
                        start=(c == 0),
                        stop=(c == c_diag + sub),
                    )

            # off-diagonal full k-tile pairs
            for g in range(c_diag // 2):
                cs = (2 * g, 2 * g + 1)
                s_ps = qk_psum.tile([128, 1024], F32, tag="qk")
                for j, c in enumerate(cs):
                    nc.tensor.matmul(
                        s_ps[:, j * 512:(j + 1) * 512],
                        lhsT=tT_flat[:, c * KT:(c + 1) * KT],
                        rhs=qT_flat[:, q0:q0 + QB],
                    )
                num = num_pool.tile([128, 1024], BF16)
                nc.scalar.activation(num[:], s_ps[:],
                                     mybir.ActivationFunctionType.Exp,
                                     scale=SCALE)
                nc.vector.tensor_scalar_max(num[:], num[:], 1.0)
                for j, c in enumerate(cs):
                    pv(c, num[:, j * 512:(j + 1) * 512], (0, 1, 2, 3))
                if hooks:
                    hooks.pop(0)()

            # diagonal block: k-tiles c_diag+j, trimmed to 512-128j columns,
            # packed in two PSUM groups: (j=0: 512, j=1: 384) and
            # (j=2: 256, j=3: 128).
            for grp, js in enumerate(((0, 1), (2, 3))):
                widths = [QB - 128 * j for j in js]
                s_ps = qk_psum.tile([128, 1024], F32, tag="qk")
                off = 0
                offs = []
                for j, w in zip(js, widths):
                    nc.tensor.matmul(
                        s_ps[:, off:off + w],
                        lhsT=tT_flat[:, (c_diag + j) * KT:(c_diag + j + 1) * KT],
                        rhs=qT_flat[:, q0 + 128 * j:q0 + QB],
                    )
                    offs.append(off)
                    off += w
                num = num_pool.tile([128, 1024], BF16)
                nc.scalar.activation(num[:, 0:off], s_ps[:, 0:off],
                                     mybir.ActivationFunctionType.Exp,
                                     scale=SCALE)
                nc.vector.tensor_scalar_max(num[:, 0:off], num[:, 0:off], 1.0)
                # causal wedge: first 128 computed cols of each diagonal tile
                for j, o in zip(js, offs):
                    nc.gpsimd.affine_select(
                        out=num[:, o:o + 128],
                        in_=num[:, o:o + 128],
                        compare_op=mybir.AluOpType.is_ge,
                        fill=0.0,
                        base=0,
                        channel_multiplier=-1,
                        pattern=[[1, 128]],
                    )
                for j, o, w in zip(js, offs, widths):
                    pv(c_diag + j, num[:, o:o + w], tuple(range(j, 4)))
                if hooks:
                    hooks.pop(0)()

            # ---- normalize: PSUM->SBUF copy (DVE), pv/den on GpSimd ----
            stg = stg_pool.tile([128, 4, 129], F32, tag="stg")
            nc.vector.tensor_copy(stg[:], obank[:, :, 0:129])
            for sub in range(4):
                nc.gpsimd.normalize_recip(
                    fin[:, (qb % 2) * 4 + sub, :],
                    stg[:, sub, 0:D],
                    stg[:, sub, D:D + 1],
                )

        alloc_prep(1)
        for b in range(B_LOC):
            for qb in range(N_QB):
                if qb % 2 == 0:
                    fin = fin_pool.tile([128, 8, 128], F32, tag="fin")
                if b == 0 and qb == 3:
                    hooks = b1_hooks_qb3
                elif b == 1 and qb == 0:
                    hooks = b1_hooks_late[:2]
                elif b == 1 and qb == 1:
                    hooks = b1_hooks_late[2:]
                else:
                    hooks = ()
                emit_qb(b, qb, fin, hooks)
                if qb % 2 == 1:
                    q0 = (qb - 1) * QB
                    nc.sync.dma_start(
                        o_ext[b, q0:q0 + 2 * QB, :].rearrange(
                            "(s p) d -> p s d", p=128),
                        fin[:])
                # batch-1 loads issued early from the GpSimd queue
                if b == 0 and qb == 0:
                    emit_b1_loads()

    nc.compile()
    return nc


_NC_CACHE = None


def _get_nc():
    global _NC_CACHE
    if _NC_CACHE is None:
        _NC_CACHE = build_attention_core()
    return _NC_CACHE


def kernel(Q: np.ndarray, T: np.ndarray, V: np.ndarray) -> np.ndarray:
    """Full-input entry point: shard over batch, run 8-core SPMD, gather."""
    from concourse.bass_utils import run_bass_kernel_spmd

    Q = np.ascontiguousarray(np.asarray(Q, dtype=np.float32))
    T = np.ascontiguousarray(np.asarray(T, dtype=np.float32))
    V = np.ascontiguousarray(np.asarray(V, dtype=np.float32))
    assert Q.shape == (B, S, D), Q.shape

    nc = _get_nc()
    in_maps = [
        {
            "Q": Q[i * B_LOC:(i + 1) * B_LOC],
            "T": T[i * B_LOC:(i + 1) * B_LOC],
            "V": V[i * B_LOC:(i + 1) * B_LOC],
        }
        for i in range(N_CORES)
    ]
    res = run_bass_kernel_spmd(nc, in_maps, core_ids=list(range(N_CORES)))
    return np.concatenate([res.results[i]["out"] for i in range(N_CORES)], axis=0)


# revision 26
# speedup vs baseline: 1.1201x; 1.0018x over previous
"""Sparse-attention Trainium2 kernel (nn_AttentionLayer, B=16 S=2048 D=128).

reference semantics:
    A = Q @ T^T                     # [B,S,S]
    A = where(A > 0.3, A, 0)
    A += where(strictly_upper, -2^32, 0)
    y = softmax(A / sqrt(D)) @ V

Sharding: data-parallel over batch, 2 batches per core on 8 NeuronCores.
No collectives.

v2 design (per core, per batch):
  - Loads split in chunks and issued up front; prep (PE transposes of
    Q/T into [128,1024] PSUM groups + one big f32->bf16 cast-copy per
    group on DVE) is pipelined under the main loop; batch-1 prep is
    emitted inside batch-0's qb loop so PE/DVE never idle.
  - Scores S^T[k,q] via matmul(lhsT=tT tile, rhs=qT block) in bf16.
    Exact-causal: off-diagonal k-tile pairs in [128,1024] PSUM groups;
    the 4 diagonal k-tiles are column-trimmed (512/384/256/128 cols).
  - num = max(exp(S^T/sqrt(d)), 1): exp on ScalarE (scale fused),
    max on DVE in bf16 (4x perf mode). Equals the reference's
    threshold-then-exp except on scores in (0,0.3], error <=2.7% of
    one softmax term.
  - Causal wedge: one [128,128] affine_select per diagonal k-tile on
    GpSimd (fill=0), only on the 128 columns straddling the diagonal.
  - PV + denominator fused: lhsT = num chunk [k,128q], rhs =
    [V | ones] [k,129] bf16, accumulated in PSUM over k. obanks packed
    2 subtiles per PSUM tile [128,2,129]. Denominator at column 128.
  - Normalize: obank pair copied PSUM->SBUF on DVE, then
    normalize_recip (out = pv/den) on GpSimd. Stores 1 per 2 q-blocks.
"""

from contextlib import ExitStack

import numpy as np

import concourse.bass as bass
import concourse.mybir as mybir
import concourse.tile as tile
from concourse import bacc

B, S, D = 16, 2048, 128
N_CORES = 8
B_LOC = B // N_CORES          # 2 batches per core
QB = 512                      # q-block width (matmul moving dim)
KT = 128                      # k-tile height (partition dim)
N_QB = S // QB                # 4 q-blocks
N_ST = S // 128               # 16 seq tiles
SCALE = float(1.0 / np.sqrt(D))

F32 = mybir.dt.float32
BF16 = mybir.dt.bfloat16


def build_attention_core():
    """Build the single-core SPMD graph: [B_LOC,S,D] Q/T/V -> [B_LOC,S,D] out."""
    from concourse.masks import make_identity

    nc = bacc.Bacc("TRN2", target_bir_lowering=False, debug=False,
                   num_devices=N_CORES)
    q_ext = nc.dram_tensor("Q", [B_LOC, S, D], F32, kind="ExternalInput").ap()
    t_ext = nc.dram_tensor("T", [B_LOC, S, D], F32, kind="ExternalInput").ap()
    v_ext = nc.dram_tensor("V", [B_LOC, S, D], F32, kind="ExternalInput").ap()
    o_ext = nc.dram_tensor("out", [B_LOC, S, D], F32, kind="ExternalOutput").ap()

    with tile.TileContext(nc) as tc, ExitStack() as ctx:
        nat_pool = ctx.enter_context(tc.tile_pool(name="nat", bufs=1))
        qt_pool = ctx.enter_context(tc.tile_pool(name="qt", bufs=1))
        tt_pool = ctx.enter_context(tc.tile_pool(name="tt", bufs=1))
        vb_pool = ctx.enter_context(tc.tile_pool(name="vb", bufs=1))
        num_pool = ctx.enter_context(tc.tile_pool(name="num", bufs=4))
        stg_pool = ctx.enter_context(tc.tile_pool(name="stg", bufs=2))
        fin_pool = ctx.enter_context(tc.tile_pool(name="fin", bufs=2))
        const_pool = ctx.enter_context(tc.tile_pool(name="const", bufs=1))
        # PSUM: qk pool tiles [128,1024] f32 = 2 banks x2 bufs = 4 banks;
        # out pool tiles [128,2,129] f32 = 1 bank x4 bufs = 4 banks.
        qk_psum = ctx.enter_context(tc.tile_pool(name="qk_ps", bufs=2, space="PSUM"))
        out_psum = ctx.enter_context(tc.tile_pool(name="out_ps", bufs=1, space="PSUM"))

        ident = const_pool.tile([128, 128], F32)
        make_identity(nc, ident[:])

        # ---- issue ALL input loads up front (b0 in halves for fast start) ----
        nats = []   # per batch: (q_nat, t_nat, v_nat)
        for b in range(B_LOC):
            q_nat = nat_pool.tile([128, N_ST, D], F32, name=f"q_nat{b}")
            t_nat = nat_pool.tile([128, N_ST, D], F32, name=f"t_nat{b}")
            v_nat = nat_pool.tile([128, N_ST, D], F32, name=f"v_nat{b}")
            nats.append((q_nat, t_nat, v_nat))
        # Loads are issue-parallelized across the three DMA-capable queues
        # (Sync: T, Scalar: Q, GpSimd: V) and chunked so the tiles needed by
        # q-block 0 (T/Q/V tiles 0:4) hit the DMA rings first. Batch-1 loads
        # are issued later from the GpSimd queue (staggered in the main loop)
        # so they don't steal ring bandwidth from batch 0's critical chunks.
        exts = [(t_ext, 1), (q_ext, 0), (v_ext, 2)]

        def load(b, eng, ei, t0, t1):
            ext, which = exts[ei]
            eng.dma_start(
                nats[b][which][:, t0:t1, :],
                ext[b].rearrange("(t p) d -> p t d", p=128)[:, t0:t1, :])

        for t0, t1 in ((0, 4), (4, 8), (8, 16)):
            load(0, nc.sync, 0, t0, t1)      # T chunks
            load(0, nc.scalar, 1, t0, t1)    # Q chunks
            load(0, nc.gpsimd, 2, t0, t1)    # V chunks

        # ---- prep helpers ------------------------------------------------
        qTs, tTs, v_augs = {}, {}, {}

        def alloc_prep(b):
            qTs[b] = qt_pool.tile([128, N_ST, 128], BF16, name=f"qT{b}")
            tTs[b] = tt_pool.tile([128, N_ST, 128], BF16, name=f"tT{b}")
            v_augs[b] = vb_pool.tile([128, N_ST, 132], BF16, name=f"vaug{b}")

        def emit_transpose_group(b, which, t0, t1):
            """Transpose tiles [t0, t1) of Q (which=0) or T (which=1) of
            batch b into one PSUM group, then one f32->bf16 cast-copy to the
            persistent qT/tT tile."""
            src = nats[b][which]
            dst = (qTs if which == 0 else tTs)[b]
            n = t1 - t0
            ps = qk_psum.tile([128, 1024], F32, tag="qk")
            for i in range(n):
                nc.tensor.transpose(ps[:, i * 128:(i + 1) * 128],
                                    src[:, t0 + i, :], ident[:])
            nc.vector.tensor_copy(dst[:, t0:t1, :], ps[:, 0:n * 128])

        def emit_vaug(b, t0, t1):
            """Cast V tiles [t0, t1) to bf16 into v_aug (DVE)."""
            nc.vector.tensor_copy(v_augs[b][:, t0:t1, 0:D],
                                  nats[b][2][:, t0:t1, :])

        # ---- batch-0 prep (chunks ordered to unblock q-block 0 fastest) ----
        alloc_prep(0)
        emit_transpose_group(0, 1, 0, 4)    # T tiles 0:4
        emit_transpose_group(0, 0, 0, 4)    # Q tiles 0:4
        emit_vaug(0, 0, 4)
        nc.gpsimd.memset(v_augs[0][:, :, D:D + 1], 1.0)
        emit_transpose_group(0, 1, 4, 8)
        emit_transpose_group(0, 0, 4, 8)
        emit_vaug(0, 4, 8)
        emit_transpose_group(0, 1, 8, 16)
        emit_transpose_group(0, 0, 8, 16)
        emit_vaug(0, 8, 16)

        # deferred batch-1 loads (GpSimd queue) and prep, interleaved into
        # batch 0's later q-blocks (each hook is one small unit of work so no
        # single insertion starves the exp pipeline)
        def b1_loads_a():
            load(1, nc.gpsimd, 0, 0, 8)
            load(1, nc.gpsimd, 1, 0, 8)

        def b1_loads_b():
            load(1, nc.gpsimd, 0, 8, 16)
            load(1, nc.gpsimd, 1, 8, 16)
            load(1, nc.gpsimd, 2, 0, 16)

        b1_hooks_qb3 = [
            lambda: emit_transpose_group(1, 1, 0, 4),
            lambda: emit_transpose_group(1, 0, 0, 4),
            lambda: (emit_vaug(1, 0, 4),
                     nc.gpsimd.memset(v_augs[1][:, :, D:D + 1], 1.0)),
            lambda: emit_transpose_group(1, 1, 4, 8),
            lambda: emit_transpose_group(1, 0, 4, 8),
            lambda: emit_vaug(1, 4, 8),
        ]
        b1_hooks_late = [
            lambda: emit_transpose_group(1, 1, 8, 16),
            lambda: emit_transpose_group(1, 0, 8, 16),
            lambda: emit_vaug(1, 8, 16),
        ]

        # ---- main attention loops ----
        def emit_qb(b, qb, fin, hooks=()):
            """Emit one q-block: scores -> exp/max/select -> PV -> normalize.
            fin: [128, 8, 128] staging tile for 2 q-blocks; this qb uses
            slot (qb % 2). hooks: thunks emitted one-per-score-group to
            interleave prep work for the next batch."""
            hooks = list(hooks)
            qT_flat = qTs[b][:].rearrange("p t q -> p (t q)")
            tT_flat = tTs[b][:].rearrange("p t k -> p (t k)")
            v_aug = v_augs[b]
            q0 = qb * QB
            c_diag = 4 * qb              # first diagonal k-tile index

            # one PSUM tile, 4 bank-aligned sub-accumulators [128, 129 of 512]
            obank = out_psum.tile([128, 4, 512], F32, tag="ob")

            def pv(c, num_ap, subs):
                """PV matmuls for k-tile c; num_ap[:, i*128:(i+1)*128] is the
                numerator chunk for sub subs[i]."""
                for i, sub in enumerate(subs):
                    nc.tensor.matmul(
                        obank[:, sub, 0:129],
                        lhsT=num_ap[:, i * 128:(i + 1) * 128],
                        rhs=v_aug[:, c, 0:129],
                        start=(c == 0),
                        stop=(c == c_diag + sub),
                    )

            # off-diagonal full k-tile pairs
            for g in range(c_diag // 2):
                cs = (2 * g, 2 * g + 1)
                s_ps = qk_psum.tile([128, 1024], F32, tag="qk")
                for j, c in enumerate(cs):
                    nc.tensor.matmul(
                        s_ps[:, j * 512:(j + 1) * 512],
                        lhsT=tT_flat[:, c * KT:(c + 1) * KT],
                        rhs=qT_flat[:, q0:q0 + QB],
                    )
                num = num_pool.tile([128, 1024], BF16)
                nc.scalar.activation(num[:], s_ps[:],
                                     mybir.ActivationFunctionType.Exp,
                                     scale=SCALE)
                nc.vector.tensor_scalar_max(num[:], num[:], 1.0)
                for j, c in enumerate(cs):
                    pv(c, num[:, j * 512:(j + 1) * 512], (0, 1, 2, 3))
                if hooks:
                    hooks.pop(0)()

            # diagonal block: k-tiles c_diag+j, trimmed to 512-128j columns,
            # packed in two PSUM groups: (j=0: 512, j=1: 384) and
            # (j=2: 256, j=3: 128).
            for grp, js in enumerate(((0, 1), (2, 3))):
                widths = [QB - 128 * j for j in js]
                s_ps = qk_psum.tile([128, 1024], F32, tag="qk")
                off = 0
                offs = []
                for j, w in zip(js, widths):
                    nc.tensor.matmul(
                        s_ps[:, off:off + w],
                        lhsT=tT_flat[:, (c_diag + j) * KT:(c_diag + j + 1) * KT],
                        rhs=qT_flat[:, q0 + 128 * j:q0 + QB],
                    )
                    offs.append(off)
                    off += w
                num = num_pool.tile([128, 1024], BF16)
                nc.scalar.activation(num[:, 0:off], s_ps[:, 0:off],
                                     mybir.ActivationFunctionType.Exp,
                                     scale=SCALE)
                nc.vector.tensor_scalar_max(num[:, 0:off], num[:, 0:off], 1.0)
                # causal wedge: first 128 computed cols of each diagonal tile
                for j, o in zip(js, offs):
                    nc.gpsimd.affine_select(
                        out=num[:, o:o + 128],
                        in_=num[:, o:o + 128],
                        compare_op=mybir.AluOpType.is_ge,
                        fill=0.0,
                        base=0,
                        channel_multiplier=-1,
                        pattern=[[1, 128]],
                    )
                for j, o, w in zip(js, offs, widths):
                    pv(c_diag + j, num[:, o:o + w], tuple(range(j, 4)))
                if hooks:
                    hooks.pop(0)()

            # ---- normalize: PSUM->SBUF copy (DVE), pv/den on GpSimd ----
            stg = stg_pool.tile([128, 4, 129], F32, tag="stg")
            nc.vector.tensor_copy(stg[:], obank[:, :, 0:129])
            for sub in range(4):
                nc.gpsimd.normalize_recip(
                    fin[:, (qb % 2) * 4 + sub, :],
                    stg[:, sub, 0:D],
                    stg[:, sub, D:D + 1],
                )

        alloc_prep(1)
        for b in range(B_LOC):
            for qb in range(N_QB):
                if qb % 2 == 0:
                    fin = fin_pool.tile([128, 8, 128], F32, tag="fin")
                if b == 0 and qb == 3:
                    hooks = b1_hooks_qb3
                elif b == 1 and qb == 0:
                    hooks = b1_hooks_late[:2]
                elif b == 1 and qb == 1:
                    hooks = b1_hooks_late[2:]
                else:
                    hooks = ()
                emit_qb(b, qb, fin, hooks)
                if qb % 2 == 1:
                    q0 = (qb - 1) * QB
                    nc.sync.dma_start(
                        o_ext[b, q0:q0 + 2 * QB, :].rearrange(
                            "(s p) d -> p s d", p=128),
                        fin[:])
                # batch-1 loads, staggered so they don't contend with the
                # batch-0 chunks still in flight
                if b == 0 and qb == 1:
                    b1_loads_a()
                if b == 0 and qb == 2:
                    b1_loads_b()

    nc.compile()
    return nc


_NC_CACHE = None


def _get_nc():
    global _NC_CACHE
    if _NC_CACHE is None:
        _NC_CACHE = build_attention_core()
    return _NC_CACHE


def kernel(Q: np.ndarray, T: np.ndarray, V: np.ndarray) -> np.ndarray:
    """Full-input entry point: shard over batch, run 8-core SPMD, gather."""
    from concourse.bass_utils import run_bass_kernel_spmd

    Q = np.ascontiguousarray(np.asarray(Q, dtype=np.float32))
    T = np.ascontiguousarray(np.asarray(T, dtype=np.float32))
    V = np.ascontiguousarray(np.asarray(V, dtype=np.float32))
    assert Q.shape == (B, S, D), Q.shape

    nc = _get_nc()
    in_maps = [
        {
            "Q": Q[i * B_LOC:(i + 1) * B_LOC],
            "T": T[i * B_LOC:(i + 1) * B_LOC],
            "V": V[i * B_LOC:(i + 1) * B_LOC],
        }
        for i in range(N_CORES)
    ]
    res = run_bass_kernel_spmd(nc, in_maps, core_ids=list(range(N_CORES)))
    return np.concatenate([res.results[i]["out"] for i in range(N_CORES)], axis=0)


# revision 32
# speedup vs baseline: 1.2482x; 1.1144x over previous
"""Sparse-attention Trainium2 kernel (nn_AttentionLayer, B=16 S=2048 D=128).

reference semantics:
    A = Q @ T^T                     # [B,S,S]
    A = where(A > 0.3, A, 0)
    A += where(strictly_upper, -2^32, 0)
    y = softmax(A / sqrt(D)) @ V

Sharding: data-parallel over batch, 2 batches per core on 8 NeuronCores.
No collectives.

v2 design (per core, per batch):
  - Loads split in chunks and issued up front; prep (PE transposes of
    Q/T into [128,1024] PSUM groups + one big f32->bf16 cast-copy per
    group on DVE) is pipelined under the main loop; batch-1 prep is
    emitted inside batch-0's qb loop so PE/DVE never idle.
  - Scores S^T[k,q] via matmul(lhsT=tT tile, rhs=qT block) in bf16.
    Exact-causal: off-diagonal k-tile pairs in [128,1024] PSUM groups;
    the 4 diagonal k-tiles are column-trimmed (512/384/256/128 cols).
  - num = max(exp(S^T/sqrt(d)), 1): exp on ScalarE (scale fused),
    max on DVE in bf16 (4x perf mode). Equals the reference's
    threshold-then-exp except on scores in (0,0.3], error <=2.7% of
    one softmax term.
  - Causal wedge: one [128,128] affine_select per diagonal k-tile on
    GpSimd (fill=0), only on the 128 columns straddling the diagonal.
  - PV + denominator fused: lhsT = num chunk [k,128q], rhs =
    [V | ones] [k,129] bf16, accumulated in PSUM over k. obanks packed
    2 subtiles per PSUM tile [128,2,129]. Denominator at column 128.
  - Normalize: obank pair copied PSUM->SBUF on DVE, then
    normalize_recip (out = pv/den) on GpSimd. Stores 1 per 2 q-blocks.
"""

from contextlib import ExitStack

import numpy as np

import concourse.bass as bass
import concourse.mybir as mybir
import concourse.tile as tile
from concourse import bacc

B, S, D = 16, 2048, 128
N_CORES = 8
B_LOC = B // N_CORES          # 2 batches per core
QB = 512                      # q-block width (matmul moving dim)
KT = 128                      # k-tile height (partition dim)
N_QB = S // QB                # 4 q-blocks
N_ST = S // 128               # 16 seq tiles
SCALE = float(1.0 / np.sqrt(D))

F32 = mybir.dt.float32
BF16 = mybir.dt.bfloat16


def build_attention_core():
    """Build the single-core SPMD graph: [B_LOC,S,D] Q/T/V -> [B_LOC,S,D] out."""
    from concourse.masks import make_identity

    nc = bacc.Bacc("TRN2", target_bir_lowering=False, debug=False,
                   num_devices=N_CORES)
    q_ext = nc.dram_tensor("Q", [B_LOC, S, D], F32, kind="ExternalInput").ap()
    t_ext = nc.dram_tensor("T", [B_LOC, S, D], F32, kind="ExternalInput").ap()
    v_ext = nc.dram_tensor("V", [B_LOC, S, D], F32, kind="ExternalInput").ap()
    o_ext = nc.dram_tensor("out", [B_LOC, S, D], F32, kind="ExternalOutput").ap()

    with tile.TileContext(nc) as tc, ExitStack() as ctx:
        nat_pool = ctx.enter_context(tc.tile_pool(name="nat", bufs=1))
        qt_pool = ctx.enter_context(tc.tile_pool(name="qt", bufs=1))
        tt_pool = ctx.enter_context(tc.tile_pool(name="tt", bufs=1))
        vb_pool = ctx.enter_context(tc.tile_pool(name="vb", bufs=1))
        num_pool = ctx.enter_context(tc.tile_pool(name="num", bufs=4))
        stg_pool = ctx.enter_context(tc.tile_pool(name="stg", bufs=2))
        fin_pool = ctx.enter_context(tc.tile_pool(name="fin", bufs=2))
        const_pool = ctx.enter_context(tc.tile_pool(name="const", bufs=1))
        # PSUM: qk pool tiles [128,1024] f32 = 2 banks x2 bufs = 4 banks;
        # out pool tiles [128,2,129] f32 = 1 bank x4 bufs = 4 banks.
        qk_psum = ctx.enter_context(tc.tile_pool(name="qk_ps", bufs=2, space="PSUM"))
        out_psum = ctx.enter_context(tc.tile_pool(name="out_ps", bufs=1, space="PSUM"))

        ident = const_pool.tile([128, 128], F32)
        make_identity(nc, ident[:])

        # nat tiles are SHARED between the two batches: batch 1's loads
        # overwrite batch 0's tiles, so the WAR dependency (b0's transposes /
        # v_aug casts must finish reading first) naturally staggers b1's DMA
        # traffic behind b0's critical chunks on the rings.
        q_nat = nat_pool.tile([128, N_ST, D], F32, name="q_nat")
        t_nat = nat_pool.tile([128, N_ST, D], F32, name="t_nat")
        v_nat = nat_pool.tile([128, N_ST, D], F32, name="v_nat")
        nats = [(q_nat, t_nat, v_nat)] * B_LOC
        # Loads are issue-parallelized across the three DMA-capable queues
        # (Sync: T, Scalar: Q, GpSimd: V) and chunked so the tiles needed by
        # q-block 0 (T/Q/V tiles 0:4) hit the DMA rings first. Batch-1 loads
        # are issued later from the GpSimd queue (staggered in the main loop)
        # so they don't steal ring bandwidth from batch 0's critical chunks.
        exts = [(t_ext, 1), (q_ext, 0), (v_ext, 2)]

        def load(b, eng, ei, t0, t1):
            ext, which = exts[ei]
            eng.dma_start(
                nats[b][which][:, t0:t1, :],
                ext[b].rearrange("(t p) d -> p t d", p=128)[:, t0:t1, :])

        for t0, t1 in ((0, 4), (4, 8), (8, 16)):
            load(0, nc.sync, 0, t0, t1)      # T chunks
            load(0, nc.scalar, 1, t0, t1)    # Q chunks
            load(0, nc.gpsimd, 2, t0, t1)    # V chunks

        def b1_loads():
            # batch-1 loads: all on Sync, emitted after batch 0's last nat
            # readers so the WAR deps on the shared tiles stagger them behind
            # batch 0's chunks on the DMA rings
            for t0, t1 in ((0, 8), (8, 16)):
                for ei in range(3):
                    load(1, nc.sync, ei, t0, t1)

        # ---- prep helpers ------------------------------------------------
        qTs, tTs, v_augs = {}, {}, {}

        def alloc_prep(b):
            qTs[b] = qt_pool.tile([128, N_ST, 128], BF16, name=f"qT{b}")
            tTs[b] = tt_pool.tile([128, N_ST, 128], BF16, name=f"tT{b}")
            v_augs[b] = vb_pool.tile([128, N_ST, 132], BF16, name=f"vaug{b}")

        def emit_transpose_group(b, which, t0, t1):
            """Transpose tiles [t0, t1) of Q (which=0) or T (which=1) of
            batch b into one PSUM group, then one f32->bf16 cast-copy to the
            persistent qT/tT tile."""
            src = nats[b][which]
            dst = (qTs if which == 0 else tTs)[b]
            n = t1 - t0
            ps = qk_psum.tile([128, 1024], F32, tag="qk")
            for i in range(n):
                nc.tensor.transpose(ps[:, i * 128:(i + 1) * 128],
                                    src[:, t0 + i, :], ident[:])
            nc.vector.tensor_copy(dst[:, t0:t1, :], ps[:, 0:n * 128])

        def emit_vaug(b, t0, t1):
            """Cast V tiles [t0, t1) to bf16 into v_aug (DVE)."""
            nc.vector.tensor_copy(v_augs[b][:, t0:t1, 0:D],
                                  nats[b][2][:, t0:t1, :])

        # ---- batch-0 prep: only q-block 0's needs up front; the rest is
        # interleaved between score groups via hooks so the PE FIFO never
        # queues transposes ahead of ready score matmuls. ----
        alloc_prep(0)
        alloc_prep(1)
        emit_transpose_group(0, 1, 0, 4)    # T tiles 0:4
        emit_transpose_group(0, 0, 0, 4)    # Q tiles 0:4
        emit_vaug(0, 0, 4)
        nc.gpsimd.memset(v_augs[0][:, :, D:D + 1], 1.0)

        hooks_by_point = {
            (0, 0): [
                lambda: emit_transpose_group(0, 1, 4, 8),
                lambda: (emit_transpose_group(0, 0, 4, 8),
                         emit_vaug(0, 4, 8)),
            ],
            (0, 1): [
                lambda: emit_transpose_group(0, 1, 8, 16),
                lambda: emit_transpose_group(0, 0, 8, 16),
                lambda: (emit_vaug(0, 8, 16), b1_loads()),
            ],
            (0, 3): [
                lambda: emit_transpose_group(1, 1, 0, 4),
                lambda: emit_transpose_group(1, 0, 0, 4),
                lambda: (emit_vaug(1, 0, 4),
                         nc.gpsimd.memset(v_augs[1][:, :, D:D + 1], 1.0)),
                lambda: emit_transpose_group(1, 1, 4, 8),
                lambda: emit_transpose_group(1, 0, 4, 8),
                lambda: emit_vaug(1, 4, 8),
            ],
            (1, 0): [
                lambda: emit_transpose_group(1, 1, 8, 16),
                lambda: emit_transpose_group(1, 0, 8, 16),
            ],
            (1, 1): [
                lambda: emit_vaug(1, 8, 16),
            ],
        }

        # ---- main attention loops ----
        def emit_qb(b, qb, fin, hooks=()):
            """Emit one q-block: scores -> exp/max/select -> PV -> normalize.
            fin: [128, 8, 128] staging tile for 2 q-blocks; this qb uses
            slot (qb % 2). hooks: thunks emitted one-per-score-group to
            interleave prep work for the next batch."""
            hooks = list(hooks)
            qT_flat = qTs[b][:].rearrange("p t q -> p (t q)")
            tT_flat = tTs[b][:].rearrange("p t k -> p (t k)")
            v_aug = v_augs[b]
            q0 = qb * QB
            c_diag = 4 * qb              # first diagonal k-tile index

            # one PSUM tile, 4 bank-aligned sub-accumulators [128, 129 of 512]
            obank = out_psum.tile([128, 4, 512], F32, tag="ob")

            def pv(c, num_ap, subs):
                """PV matmuls for k-tile c; num_ap[:, i*128:(i+1)*128] is the
                numerator chunk for sub subs[i]."""
                for i, sub in enumerate(subs):
                    nc.tensor.matmul(
                        obank[:, sub, 0:129],
                        lhsT=num_ap[:, i * 128:(i + 1) * 128],
                        rhs=v_aug[:, c, 0:129],
                        start=(c == 0),
                        stop=(c == c_diag + sub),
                    )

            # off-diagonal full k-tile pairs
            for g in range(c_diag // 2):
                cs = (2 * g, 2 * g + 1)
                s_ps = qk_psum.tile([128, 1024], F32, tag="qk")
                for j, c in enumerate(cs):
                    nc.tensor.matmul(
                        s_ps[:, j * 512:(j + 1) * 512],
                        lhsT=tT_flat[:, c * KT:(c + 1) * KT],
                        rhs=qT_flat[:, q0:q0 + QB],
                    )
                num = num_pool.tile([128, 1024], BF16)
                nc.scalar.activation(num[:], s_ps[:],
                                     mybir.ActivationFunctionType.Exp,
                                     scale=SCALE)
                nc.vector.tensor_scalar_max(num[:], num[:], 1.0)
                for j, c in enumerate(cs):
                    pv(c, num[:, j * 512:(j + 1) * 512], (0, 1, 2, 3))
                if hooks:
                    hooks.pop(0)()

            # diagonal block: k-tiles c_diag+j, trimmed to 512-128j columns,
            # packed in two PSUM groups: (j=0: 512, j=1: 384) and
            # (j=2: 256, j=3: 128).
            for grp, js in enumerate(((0, 1), (2, 3))):
                widths = [QB - 128 * j for j in js]
                s_ps = qk_psum.tile([128, 1024], F32, tag="qk")
                off = 0
                offs = []
                for j, w in zip(js, widths):
                    nc.tensor.matmul(
                        s_ps[:, off:off + w],
                        lhsT=tT_flat[:, (c_diag + j) * KT:(c_diag + j + 1) * KT],
                        rhs=qT_flat[:, q0 + 128 * j:q0 + QB],
                    )
                    offs.append(off)
                    off += w
                num = num_pool.tile([128, 1024], BF16)
                nc.scalar.activation(num[:, 0:off], s_ps[:, 0:off],
                                     mybir.ActivationFunctionType.Exp,
                                     scale=SCALE)
                nc.vector.tensor_scalar_max(num[:, 0:off], num[:, 0:off], 1.0)
                # causal wedge: first 128 computed cols of each diagonal tile
                for j, o in zip(js, offs):
                    nc.gpsimd.affine_select(
                        out=num[:, o:o + 128],
                        in_=num[:, o:o + 128],
                        compare_op=mybir.AluOpType.is_ge,
                        fill=0.0,
                        base=0,
                        channel_multiplier=-1,
                        pattern=[[1, 128]],
                    )
                for j, o, w in zip(js, offs, widths):
                    pv(c_diag + j, num[:, o:o + w], tuple(range(j, 4)))
                if hooks:
                    hooks.pop(0)()

            # ---- normalize: PSUM->SBUF copy (DVE), pv/den on GpSimd ----
            stg = stg_pool.tile([128, 4, 129], F32, tag="stg")
            nc.vector.tensor_copy(stg[:], obank[:, :, 0:129])
            for sub in range(4):
                nc.gpsimd.normalize_recip(
                    fin[:, (qb % 2) * 4 + sub, :],
                    stg[:, sub, 0:D],
                    stg[:, sub, D:D + 1],
                )

        for b in range(B_LOC):
            for qb in range(N_QB):
                if qb % 2 == 0:
                    fin = fin_pool.tile([128, 8, 128], F32, tag="fin")
                emit_qb(b, qb, fin, hooks_by_point.get((b, qb), ()))
                if qb % 2 == 1:
                    q0 = (qb - 1) * QB
                    nc.sync.dma_start(
                        o_ext[b, q0:q0 + 2 * QB, :].rearrange(
                            "(s p) d -> p s d", p=128),
                        fin[:])

    nc.compile()
    return nc


_NC_CACHE = None


def _get_nc():
    global _NC_CACHE
    if _NC_CACHE is None:
        _NC_CACHE = build_attention_core()
    return _NC_CACHE


def kernel(Q: np.ndarray, T: np.ndarray, V: np.ndarray) -> np.ndarray:
    """Full-input entry point: shard over batch, run 8-core SPMD, gather."""
    from concourse.bass_utils import run_bass_kernel_spmd

    Q = np.ascontiguousarray(np.asarray(Q, dtype=np.float32))
    T = np.ascontiguousarray(np.asarray(T, dtype=np.float32))
    V = np.ascontiguousarray(np.asarray(V, dtype=np.float32))
    assert Q.shape == (B, S, D), Q.shape

    nc = _get_nc()
    in_maps = [
        {
            "Q": Q[i * B_LOC:(i + 1) * B_LOC],
            "T": T[i * B_LOC:(i + 1) * B_LOC],
            "V": V[i * B_LOC:(i + 1) * B_LOC],
        }
        for i in range(N_CORES)
    ]
    res = run_bass_kernel_spmd(nc, in_maps, core_ids=list(range(N_CORES)))
    return np.concatenate([res.results[i]["out"] for i in range(N_CORES)], axis=0)
